# revision 40
# baseline (speedup 1.0000x reference)
"""CARAFE upsampling kernel for 8 Trainium2 NeuronCores — banded-GEMM version.

Reference op (per batch b):
  xc   = conv1x1(x, w1) + b1                     # (CC=64, H, W)
  mask = conv3x3(xc, w2, pad=1) + b2             # (100, H, W)
  mask = softmax over the 25 kernel taps (per q in 4 = SF*SF groups)
  out[q, c, h, w] = sum_k mask[q, k, h, w] * x[c, h+di-2, w+dj-2]
  out pixel-shuffled by SF=2 -> (C, 2H, 2W)

Sharding: 8 shards = batch(4) x H-halves(2), 32 output rows each.

Combine strategy: for each output row h and q-pair P, the 25-tap weighted
gather is a matmul contracting over the padded-w axis (68):
    psum[(qp,w), c] += sum_di sum_w' Band_{h,di,P}[w', (qp,w)] * xT[w', h+di, c]
where Band[w+dj, (qp,w)] = mask_n[q*25+di*5+dj, h, w] is a banded matrix
built from the normalized mask by a diagonal-scatter DMA, and
xT[wpad, hpad, c] is a host-provided transpose of the padded input.
All matmul operands bf16; PSUM accumulates fp32; output stored bf16.
"""

import os
from functools import lru_cache

import numpy as np
import ml_dtypes

import concourse.mybir as mybir
from concourse import bacc
import concourse.tile as tile
from concourse.bass import AP
from concourse.bass_utils import run_bass_kernel_spmd

F32 = mybir.dt.float32
BF16 = mybir.dt.bfloat16
_BF16NP = ml_dtypes.bfloat16
AF = mybir.ActivationFunctionType

# Problem constants (hardcoded; kernel.py must be self-contained).
B, C, H, W = 4, 256, 64, 64
CC = 64           # compressed channels
SF = 2            # scale factor
KA = 25           # taps
NQ = 4            # quadrants
NM = NQ * KA      # 100 mask channels

HL = 32           # local (per-shard) output rows
HP = HL + 4       # padded rows (2 halo each side)
WP2 = W + 4       # padded cols
NPIX = HL * W     # 2048 output pixels per shard
NPAD = HP * WP2   # 2448 padded pixels

BFREE = 5 * 2 * 2 * W * HL   # band free size = 40960
QHW = W * HL                 # 2048 (per-(di,q) block in band cols)

N_CORES = 8


def _scatter_band(nc, msk_T, stg_d, band):
    """Scatter msk_T[100, (w*32+h)] into the DRAM staging band image,
    then copy each di-chunk back into the SBUF band tile.

    stg[w+dj, di, P, qp, w, h] = msk_T[q*25+di*5+dj, w*32+h], q = 2P+qp.
    SBUF APs cannot express diagonals (partition stride must be a whole
    row multiple), but DRAM APs are flat - so the diagonal lives on the
    DRAM side.  All stg DMAs go on the qAct queue (nc.scalar) so
    zero-fill -> scatter(di) -> band-in(di) are ordered by queue FIFO;
    interleaving per di lets stage F start after the first chunk.
    Scatter is split by w-half and band-in by partition-chunk so
    descriptors spread across more DMA engines.
    """
    mt = msk_T[:].tensor
    st = stg_d[:].tensor
    # One DMA per (di, q): each SP-stream DMA trigger costs ~1.2us, so
    # fewer triggers beats more engine spread here.
    for di in range(5):
        for q in range(4):
            src = AP(mt, (q * 25 + di * 5) * NPIX,
                     [[NPIX, 5], [HL, W], [1, HL]])
            dst = AP(st, di * (2 * 2 * QHW) + q * QHW,
                     [[BFREE, 5], [BFREE + HL, W], [1, HL]])
            nc.sync.dma_start(dst, src)
    # band-in AFTER the whole scatter (sequential on the same queue;
    # interleaving per di serializes the queue and is much slower).
    # One DMA per (di, P) into a SEPARATE tile: each DMA instruction is
    # served by ~one engine, and same-tile writes would be WAW-chained,
    # so more tiles -> more engine parallelism.
    for di in range(5):
        for P in range(2):
            src = AP(st, di * (2 * 2 * QHW) + P * (2 * QHW),
                     [[BFREE, WP2], [1, 2 * QHW]])
            nc.sync.dma_start(band[di * 2 + P][:], src)


def _build_program():
    nc = bacc.Bacc("TRN2", target_bir_lowering=False, debug=False)

    # ---- DRAM parameters -------------------------------------------------
    xcm0_d = nc.dram_tensor("xcm0", [128, NPAD], BF16, kind="ExternalInput")
    xcm1_d = nc.dram_tensor("xcm1", [128, NPAD], BF16, kind="ExternalInput")
    xt_d = nc.dram_tensor("xt", [WP2, HP, C], BF16, kind="ExternalInput")
    w1t_d = nc.dram_tensor("w1t", [2, 128, CC], BF16, kind="ExternalInput")
    w2t_d = nc.dram_tensor("w2t", [CC, 9, NM], BF16, kind="ExternalInput")
    b1_d = nc.dram_tensor("b1v", [CC, 1], F32, kind="ExternalInput")
    b2_d = nc.dram_tensor("b2v", [NM, 1], F32, kind="ExternalInput")
    osum_d = nc.dram_tensor("osum", [NM, NQ], BF16, kind="ExternalInput")
    orep_d = nc.dram_tensor("orep", [NQ, NM], BF16, kind="ExternalInput")
    out_d = nc.dram_tensor("out", [128, HL, 2, C], BF16, kind="ExternalOutput")
    stg_d = nc.dram_tensor("stg", [WP2, BFREE], BF16, kind="Internal")

    with tile.TileContext(nc) as tc:
        with (
            tc.tile_pool(name="wpool", bufs=1) as wpool,
            tc.tile_pool(name="xpool", bufs=1) as xpool,
            tc.tile_pool(name="mpool", bufs=1) as mpool,
            tc.tile_pool(name="bandp", bufs=1) as bandp,
            tc.tile_pool(name="opool", bufs=1) as opool,
            tc.tile_pool(name="psA", bufs=2, space="PSUM") as psA,
            tc.tile_pool(name="psB", bufs=2, space="PSUM") as psB,
            tc.tile_pool(name="psO", bufs=4, space="PSUM") as psO,
        ):
            # ---- load inputs -------------------------------------------
            # Queue plan: SP queue (nc.sync) carries the stg chain
            # (zero-fill -> scatter -> band-in); SP has no compute, so
            # DMA-ring backpressure on its stream is harmless.  ACT
            # queue (nc.scalar) carries weights FIRST (tiny; the conv
            # fences need them early) then xcm.  xt rides the gpsimd
            # software-DGE queue so it overlaps both.
            # conv1x1 needs only w1 + xcm; defer the other weights so
            # xcm drains the ACT queue as early as possible.
            w1sb = wpool.tile([128, 2, CC], BF16, tag="w1sb")
            nc.scalar.dma_start(w1sb[:, 0, :], w1t_d[0])
            nc.scalar.dma_start(w1sb[:, 1, :], w1t_d[1])
            b1c = wpool.tile([CC, 1], F32, tag="b1c")
            nc.scalar.dma_start(b1c[:], b1_d[:])

            # split by column-half: with subtile deps the conv1x1 fence
            # (col 0) and early chunks unblock after the first half.
            XH = NPAD // 2  # 1224
            xcm0 = xpool.tile([128, NPAD], BF16, tag="xcm0")
            xcm1 = xpool.tile([128, NPAD], BF16, tag="xcm1")
            nc.scalar.dma_start(xcm0[:, 0:XH], xcm0_d[:, 0:XH])
            nc.scalar.dma_start(xcm1[:, 0:XH], xcm1_d[:, 0:XH])
            nc.scalar.dma_start(xcm0[:, XH:], xcm0_d[:, XH:])
            nc.scalar.dma_start(xcm1[:, XH:], xcm1_d[:, XH:])

            w2sb = wpool.tile([CC, 9, NM], BF16, tag="w2sb")
            nc.scalar.dma_start(w2sb[:], w2t_d[:])
            b2c = wpool.tile([NM, 1], F32, tag="b2c")
            nc.scalar.dma_start(b2c[:], b2_d[:])
            osum = wpool.tile([NM, NQ], BF16, tag="osum")
            nc.scalar.dma_start(osum[:], osum_d[:])
            orep = wpool.tile([NQ, NM], BF16, tag="orep")
            nc.scalar.dma_start(orep[:], orep_d[:])

            xt = xpool.tile([WP2, HP, C], BF16, tag="xt")
            nc.gpsimd.dma_start(xt[:], xt_d[:])

            band = [bandp.tile([WP2, 2, W, HL], BF16, tag=f"band{k}",
                               name=f"band{k}") for k in range(10)]

            # Zero-fill the DRAM staging image (structural zeros of the
            # band).  Off the critical path: runs on qAct during convs.
            zt = xpool.tile([128, 4096], BF16, tag="zt")
            nc.gpsimd.memset(zt[:], 0.0)
            NZT = 128 * 4096  # big chunks: SP trigger issue is the cost
            NSTG = WP2 * BFREE  # 2785280
            zoff = 0
            while zoff < NSTG:
                n = min(NZT, NSTG - zoff)
                rows = n // 4096
                dst = AP(stg_d[:].tensor, zoff, [[4096, rows], [1, 4096]])
                nc.sync.dma_start(dst, zt[0:rows, :])
                zoff += n

            # ---- PE fences on DMA'd matmul operands --------------------
            # Only what conv1x1 needs; the rest are fenced after it so
            # the PE stream is never blocked on later-arriving tiles.
            for fap in (w1sb[:, 0, 0:1], xcm0[:, 0:1], xcm1[:, 0:1]):
                psf = psA.tile([1, 1], F32, tag="psa")
                nc.tensor.matmul(psf[:], fap, fap, start=True, stop=True)

            # ---- stage A: conv1x1 over the padded grid -> xcb bf16 -----
            xcb = mpool.tile([CC, NPAD], BF16, tag="xcb")
            CHUNK = 512
            nchunks = (NPAD + CHUNK - 1) // CHUNK  # 5 (last = 400)
            for i in range(nchunks):
                n0 = i * CHUNK
                n1 = min(NPAD, n0 + CHUNK)
                ps = psA.tile([CC, CHUNK], F32, tag="psa")
                nc.tensor.matmul(ps[:, : n1 - n0], w1sb[:, 0, :],
                                 xcm0[:, n0:n1], start=True, stop=False)
                nc.tensor.matmul(ps[:, : n1 - n0], w1sb[:, 1, :],
                                 xcm1[:, n0:n1], start=False, stop=True)
                nc.vector.tensor_scalar_add(xcb[:, n0:n1], ps[:, : n1 - n0],
                                            b1c[:, 0:1])

            # fences for the tiles conv3x3/softmax need (arrive later)
            for fap in (w2sb[:, 0, 0:1], osum[:, 0:1], orep[:, 0:1]):
                psf = psA.tile([1, 1], F32, tag="psa")
                nc.tensor.matmul(psf[:], fap, fap, start=True, stop=True)

            xcb3 = xcb[:].rearrange("c (h w) -> c h w", w=WP2)

            # ---- stage B: conv3x3 -> exp(mask+b2), bf16 ----------------
            msk_e = mpool.tile([NM, HL, W], BF16, tag="msk_e")
            HR = 8
            for i in range(HL // HR):  # 4 chunks of 8 rows
                psm = psB.tile([NM, HR, W], F32, tag="psb")
                for tap in range(9):
                    dy, dx = tap // 3, tap % 3
                    rhs = xcb3[:, i * HR + 1 + dy: i * HR + 1 + dy + HR,
                               1 + dx: 1 + dx + W]
                    nc.tensor.matmul(psm[:], w2sb[:, tap, :], rhs,
                                     start=(tap == 0), stop=(tap == 8))
                nc.scalar.activation(msk_e[:, i * HR:(i + 1) * HR, :], psm[:],
                                     AF.Exp, bias=b2c[:, 0:1])

            msk_ef = msk_e[:].rearrange("m h w -> m (h w)")

            # ---- stage C: softmax denominators -> rs = 1/sum, bf16 -----
            # 1/S = exp(-ln(S)) (ACT Reciprocal is banned).  All Ln ops
            # grouped before all Exp ops to minimize ACT table-set loads.
            rs = mpool.tile([NQ, NPIX], BF16, tag="rs")
            tln = mpool.tile([NQ, NPIX], F32, tag="tln")
            for i in range(NPIX // CHUNK):
                pss = psA.tile([NQ, CHUNK], F32, tag="psa")
                nc.tensor.matmul(pss[:], osum[:],
                                 msk_ef[:, i * CHUNK:(i + 1) * CHUNK],
                                 start=True, stop=True)
                nc.scalar.activation(tln[:, i * CHUNK:(i + 1) * CHUNK],
                                     pss[:], AF.Ln)
            for i in range(NPIX // CHUNK):
                nc.scalar.activation(rs[:, i * CHUNK:(i + 1) * CHUNK],
                                     tln[:, i * CHUNK:(i + 1) * CHUNK],
                                     AF.Exp, scale=-1.0)

            # ---- stage D: normalize, TRANSPOSED write  msk_T[m, w, h] --
            msk_T = mpool.tile([NM, W, HL], BF16, tag="msk_T")
            for i in range(HL // HR):
                psr = psB.tile([NM, CHUNK], F32, tag="psb")
                nc.tensor.matmul(psr[:], orep[:],
                                 rs[:, i * CHUNK:(i + 1) * CHUNK],
                                 start=True, stop=True)
                # out iterated in (h, w) order, written at col w*HL + h
                outap = msk_T[:, :, i * HR:(i + 1) * HR].rearrange(
                    "m w h -> m h w")
                nc.vector.tensor_mul(outap, msk_e[:, i * HR:(i + 1) * HR, :],
                                     psr[:].rearrange("m (h w) -> m h w", w=W))

            # ---- stage E: scatter msk_T -> stg (DRAM), then band-in ----
            _scatter_band(nc, msk_T, stg_d, band)

            # PE fence on xt (loads last; fence here, not before convs)
            psf2 = psA.tile([1, 1], F32, tag="psa")
            nc.tensor.matmul(psf2[:], xt[:, 0, 0:1], xt[:, 0, 0:1],
                             start=True, stop=True)

            # ---- stage F: banded matmuls + copy-out --------------------
            obuf = opool.tile([128, HL, 2, C], BF16, tag="obuf")
            HS = 4  # h-stripe
            ncopy = 0
            for s in range(HL // HS):
                psos = [psO.tile([128, 2, C], F32, tag="pso", name=f"pso{s}_{j}")
                        for j in range(HS)]
                for di in range(5):
                    for hh in range(HS):
                        h = s * HS + hh
                        for P in range(2):
                            # start=True clears has_written bits for the
                            # WHOLE bank, so only the very first matmul
                            # into this tile may set it; the P=1 group
                            # then starts via cleared bits (overwrite).
                            nc.tensor.matmul(
                                psos[hh][:, P, :],
                                band[di * 2 + P][:, :, :, h],
                                xt[:, h + di, :],
                                start=(di == 0 and P == 0), stop=(di == 4),
                            )
                for hh in range(HS):
                    h = s * HS + hh
                    if ncopy % 2 == 0:
                        nc.vector.tensor_copy(obuf[:, h, :, :], psos[hh][:])
                    else:
                        nc.scalar.copy(obuf[:, h, :, :], psos[hh][:])
                    ncopy += 1
                # write out this stripe, split by partition-half across
                # both queues (RAR-only deps -> DMAs run in parallel)
                h0, h1 = s * HS, (s + 1) * HS
                for ph in range(2):
                    p0, p1 = ph * 64, (ph + 1) * 64
                    eng = nc.sync if ((s + ph) % 2 == 0) else nc.scalar
                    eng.dma_start(out_d[p0:p1, h0:h1, :, :],
                                  obuf[p0:p1, h0:h1, :, :])

    nc.compile()
    return nc


@lru_cache(maxsize=1)
def _get_program(trace_debug: bool = False):
    return _build_program()


def _host_prep(x, w1, b1, w2, b2):
    """Build per-core input maps."""
    x = np.asarray(x, np.float32)
    w1 = np.asarray(w1, np.float32)
    b1 = np.asarray(b1, np.float32).reshape(CC, 1)
    w2 = np.asarray(w2, np.float32)
    b2 = np.asarray(b2, np.float32).reshape(NM, 1)

    w1t = np.ascontiguousarray(
        w1[:, :, 0, 0].T.reshape(2, 128, CC)).astype(_BF16NP)
    w2t = np.ascontiguousarray(
        w2.transpose(1, 2, 3, 0).reshape(CC, 9, NM)).astype(_BF16NP)
    osum = np.zeros((NM, NQ), np.float32)
    for q in range(NQ):
        osum[q * KA:(q + 1) * KA, q] = 1.0
    orep = np.ascontiguousarray(osum.T).astype(_BF16NP)
    osum = osum.astype(_BF16NP)

    in_maps = []
    for s in range(N_CORES):
        b, hh = s // 2, s % 2
        h0 = hh * HL
        xpad = np.zeros((C, HP, WP2), np.float32)
        r0 = max(0, h0 - 2)
        r1 = min(H, h0 + HL + 2)
        xpad[:, (r0 - h0 + 2):(r1 - h0 + 2), 2:2 + W] = x[b, :, r0:r1, :]
        xb = xpad.astype(_BF16NP)
        in_maps.append({
            "xcm0": np.ascontiguousarray(xb[:128].reshape(128, NPAD)),
            "xcm1": np.ascontiguousarray(xb[128:].reshape(128, NPAD)),
            "xt": np.ascontiguousarray(xb.transpose(2, 1, 0)),
            "w1t": w1t,
            "w2t": w2t,
            "b1v": b1,
            "b2v": b2,
            "osum": osum,
            "orep": orep,
        })
    return in_maps


def _host_post(results):
    """Reassemble full output from per-core results."""
    out = np.empty((B, C, H * SF, W * SF), np.float32)
    for s in range(N_CORES):
        b, hh = s // 2, s % 2
        o = results[s]["out"].astype(np.float32)  # [128(qp,w), 32(h), 2(P), 256(c)]
        o = o.reshape(2, W, HL, 2, C)             # [qp, w, h, P, c]
        o = o.transpose(4, 2, 3, 1, 0).reshape(C, HL * SF, W * SF)
        out[b, :, hh * HL * SF:(hh + 1) * HL * SF, :] = o
    return out


def kernel(x, w1, b1, w2, b2):
    nc = _get_program()
    in_maps = _host_prep(x, w1, b1, w2, b2)
    res = run_bass_kernel_spmd(nc, in_maps, list(range(N_CORES)))
    return _host_post(res.results)
